# revision 4
# baseline (speedup 1.0000x reference)
"""ApproxSiLU16_FXP Trainium2 kernel (8 NeuronCores, data-parallel).

The reference computes a 16-segment piecewise-linear fixed-point
approximation of SiLU on a uniform knot grid t_k = -8 + 0.875k
(k = 0..16), with knot values y_k = round(1024*silu(t_k))/1024.
Instead of gathering from the LUT per element, this kernel
reconstructs the same piecewise-linear function analytically:

    u  = x*(8/7) + 64/7            (segment coordinate, in [0,16])
    k  = floor(u)                  (magic-constant floor: +2^23-0.5)
    fr = u - k
    out = silu(t_k) + fr*(silu(t_k+0.875) - silu(t_k))

using the ScalarEngine's Silu activation for the knot values.  This
matches the fixed-point reference to ~7e-4 relative error (the only
differences are the reference's int rounding of the LUT entries and
of the interpolation weight, all sub-LSB at 2^-10 scale).

Sharding: x is (8, 2048, 4096); core i processes batch row i.
"""

import numpy as np

from concourse import bacc, mybir
import concourse.tile as tile
from concourse.bass_utils import run_bass_kernel_spmd

F32 = mybir.dt.float32
Alu = mybir.AluOpType
Act = mybir.ActivationFunctionType

P = 128          # SBUF partitions
FD = 2048        # free dim per tile
NT = 32          # tiles per core shard: 2048*4096 = NT*P*FD
N_CORES = 8

MA = 8388607.5   # 2^23 - 0.5  (magic floor, round-to-nearest-even)
MB = -8388608.0  # -2^23
C87 = float(8.0 / 7.0)
C647 = float(64.0 / 7.0)


def _reg_const(nc, val):
    t = nc.alloc_sbuf_tensor(f"const-f32-{val}", [128, 1], F32)
    nc.gpsimd.memset(t.ap(), val)
    nc.const_aps.aps[(F32, float(val))] = t.ap()


def build():
    nc = bacc.Bacc()
    _reg_const(nc, -8.0)
    _reg_const(nc, -7.125)
    nc.all_engine_barrier()
    x_ext = nc.declare_dram_parameter("x", [NT, P, FD], F32, isOutput=False)
    o_ext = nc.declare_dram_parameter("out", [NT, P, FD], F32, isOutput=True)

    with tile.TileContext(nc) as tc, tc.tile_pool(name="p", bufs=2) as pool:
        for i in range(NT):
            xt = pool.tile([P, FD], F32, tag="xt")
            nc.sync.dma_start(xt[:], x_ext[i])
            u = pool.tile([P, FD], F32, tag="u")
            nc.vector.tensor_scalar(u[:], xt[:], C87, C647, Alu.mult, Alu.add)
            kfm = pool.tile([P, FD], F32, tag="kfm")  # 2^23 + min(floor(u),16)
            nc.vector.tensor_scalar(kfm[:], u[:], 16.0, MA, Alu.min, Alu.add)
            kz = pool.tile([P, FD], F32, tag="kz")    # 0.875*k
            nc.vector.tensor_scalar(kz[:], kfm[:], MB, 0.875, Alu.add, Alu.mult)
            nfr = pool.tile([P, FD], F32, tag="nfr")  # k - u = -frac
            nc.vector.scalar_tensor_tensor(
                nfr[:], kfm[:], MB, u[:], Alu.add, Alu.subtract
            )
            a = pool.tile([P, FD], F32, tag="a")      # silu(t_k)
            nc.scalar.activation(a[:], kz[:], Act.Silu, bias=-8.0)
            b = pool.tile([P, FD], F32, tag="b")      # silu(t_{k+1})
            nc.scalar.activation(b[:], kz[:], Act.Silu, bias=-7.125)
            t = pool.tile([P, FD], F32, tag="t")
            nc.vector.tensor_tensor(t[:], b[:], a[:], Alu.subtract)
            g = pool.tile([P, FD], F32, tag="g")      # -(b-a)*frac
            nc.vector.tensor_tensor(g[:], t[:], nfr[:], Alu.mult)
            o = pool.tile([P, FD], F32, tag="o")      # a + (b-a)*frac
            nc.vector.tensor_tensor(o[:], a[:], g[:], Alu.subtract)
            nc.sync.dma_start(o_ext[i], o[:])
    nc.compile()
    return nc


_NC_CACHE = None


def _get_nc():
    global _NC_CACHE
    if _NC_CACHE is None:
        _NC_CACHE = build()
    return _NC_CACHE


def _ensure_ntff_hook():
    """Install the antenv.axon_hooks shim so trace=True works under axon."""
    import sys
    import types

    if "antenv.axon_hooks" not in sys.modules:
        mod = types.ModuleType("antenv.axon_hooks")
        _h = [None]
        mod.set_axon_ntff_profile_hook = lambda h: _h.__setitem__(0, h)
        mod.get_axon_ntff_profile_hook = lambda: _h[0]
        sys.modules["antenv.axon_hooks"] = mod
        import antenv

        antenv.axon_hooks = mod
    import antenv.axon_hooks as ah

    if ah.get_axon_ntff_profile_hook() is None:
        from trn_agent_boot.trn_boot import _ntff_profile_via_ctypes

        h = _ntff_profile_via_ctypes("/opt/axon/libaxon_pjrt.so")
        if h is not None:
            ah.set_axon_ntff_profile_hook(h)
    # avoid cloud artifact uploads in this container
    import concourse.bass_utils as bu

    bu.upload_artifacts = lambda tmpdir: tmpdir


def _run(x, trace=False, trace_kwargs=None):
    """x: (8, 2048, 4096) float32. Returns (out, exec_time_ns|None)."""
    x = np.ascontiguousarray(np.asarray(x, dtype=np.float32))
    assert x.shape == (N_CORES, 2048, 4096), x.shape
    nc = _get_nc()
    core_ids = list(range(N_CORES))
    in_maps = [{"x": x[i].reshape(NT, P, FD)} for i in range(N_CORES)]
    kwargs = {}
    if trace:
        _ensure_ntff_hook()
        kwargs["trace"] = True
        if trace_kwargs:
            kwargs.update(trace_kwargs)
    res = run_bass_kernel_spmd(nc, in_maps, core_ids, **kwargs)
    out = np.empty((N_CORES, 2048, 4096), dtype=np.float32)
    for i in range(N_CORES):
        out[i] = res.results[i]["out"].reshape(2048, 4096)
    return out, res.exec_time_ns


def kernel(x, seg=None, silu_vals=None, **_unused):
    out, _ = _run(x, trace=False)
    return out


# revision 5
# speedup vs baseline: 1.1503x; 1.1503x over previous
"""ApproxSiLU16_FXP Trainium2 kernel (8 NeuronCores, data-parallel).

The reference computes a 16-segment piecewise-linear fixed-point
approximation of SiLU on a uniform knot grid t_k = -8 + 0.875k
(k = 0..16), with knot values y_k = round(1024*silu(t_k))/1024.
Instead of gathering from the LUT per element, this kernel
reconstructs the same piecewise-linear function analytically:

    u  = x*(8/7) + 64/7            (segment coordinate, in [0,16])
    k  = floor(u)                  (magic-constant floor: +2^23-0.5)
    fr = u - k
    out = silu(t_k) + fr*(silu(t_k+0.875) - silu(t_k))

using the ScalarEngine's Silu activation for the knot values.  This
matches the fixed-point reference to ~1e-3 relative error (the only
differences are the reference's int rounding of the LUT entries / the
interpolation weight, and bf16 rounding of the blend, all well under
the 2e-2 gate).

Engine split per tile (to balance against the ~50 MB/core DMA):
  DVE : u, kfm (fp32 tensor_scalar, 2x), nfr (fused STT), g, o (bf16 2x)
  ACT : kz = 0.875*kfm - (0.875*2^23+8) via Copy-FMA; a = silu(kz);
        b = silu(kz + 0.875)  (both written bf16)
  POOL: t = a - b  (bf16 tensor_tensor)
  out = a + (a-b)*(k-u) = a + fr*(b-a), stored bf16.

Sharding: x is (8, 2048, 4096); core i processes batch row i.
"""

import numpy as np

from concourse import bacc, mybir
import concourse.tile as tile
from concourse.bass_utils import run_bass_kernel_spmd

F32 = mybir.dt.float32
BF16 = mybir.dt.bfloat16
Alu = mybir.AluOpType
Act = mybir.ActivationFunctionType

P = 128          # SBUF partitions
FD = 2048        # free dim per tile
NT = 32          # tiles per core shard: 2048*4096 = NT*P*FD
N_CORES = 8

MA = 8388607.5   # 2^23 - 0.5  (magic floor, round-to-nearest-even)
MB = -8388608.0  # -2^23
C87 = float(8.0 / 7.0)
C647 = float(64.0 / 7.0)
KZ_BIAS = float(-(0.875 * 8388608.0 + 8.0))   # -7340040, exactly representable


def _reg_const(nc, val):
    t = nc.alloc_sbuf_tensor(f"const-f32-{val}", [128, 1], F32)
    nc.gpsimd.memset(t.ap(), val)
    nc.const_aps.aps[(F32, float(val))] = t.ap()


def build():
    nc = bacc.Bacc()
    _reg_const(nc, 0.875)
    nc.all_engine_barrier()
    x_ext = nc.declare_dram_parameter("x", [NT, P, FD], F32, isOutput=False)
    o_ext = nc.declare_dram_parameter("out", [NT, P, FD], BF16, isOutput=True)

    with tile.TileContext(nc) as tc, tc.tile_pool(name="p", bufs=3) as pool:
        for i in range(NT):
            xt = pool.tile([P, FD], F32, tag="xt")
            nc.sync.dma_start(xt[:], x_ext[i])
            # u = x*(8/7) + 64/7
            u = pool.tile([P, FD], F32, tag="u")
            nc.vector.tensor_scalar(u[:], xt[:], C87, C647, Alu.mult, Alu.add)
            # kfm = 2^23 + min(floor(u), 16)
            kfm = pool.tile([P, FD], F32, tag="kfm")
            nc.vector.tensor_scalar(kfm[:], u[:], 16.0, MA, Alu.min, Alu.add)
            # kz = 0.875*kfm - (0.875*2^23 + 8) = t_k  (ACT Copy FMA, exact)
            kz = pool.tile([P, FD], F32, tag="kz")
            nc.scalar.activation(kz[:], kfm[:], Act.Copy, bias=KZ_BIAS, scale=0.875)
            # nfr = (kfm - 2^23) - u = k - u = -fr   (bf16 out)
            nfr = pool.tile([P, FD], BF16, tag="nfr")
            nc.vector.scalar_tensor_tensor(
                nfr[:], kfm[:], MB, u[:], Alu.add, Alu.subtract
            )
            # a = silu(t_k), b = silu(t_k + 0.875)   (bf16 out)
            a = pool.tile([P, FD], BF16, tag="a")
            nc.scalar.activation(a[:], kz[:], Act.Silu)
            b = pool.tile([P, FD], BF16, tag="b")
            nc.scalar.activation(b[:], kz[:], Act.Silu, bias=0.875)
            # t = a - b   (POOL, bf16)
            t = pool.tile([P, FD], BF16, tag="t")
            nc.gpsimd.tensor_tensor(t[:], a[:], b[:], Alu.subtract)
            # g = t*nfr = (b-a)*fr   (bf16 2x)
            g = pool.tile([P, FD], BF16, tag="g")
            nc.vector.tensor_tensor(g[:], t[:], nfr[:], Alu.mult)
            # o = a + g   (bf16 2x)
            o = pool.tile([P, FD], BF16, tag="o")
            nc.vector.tensor_tensor(o[:], a[:], g[:], Alu.add)
            nc.sync.dma_start(o_ext[i], o[:])
    nc.compile()
    return nc


_NC_CACHE = None


def _get_nc():
    global _NC_CACHE
    if _NC_CACHE is None:
        _NC_CACHE = build()
    return _NC_CACHE


def _ensure_ntff_hook():
    """Install the antenv.axon_hooks shim so trace=True works under axon."""
    import sys
    import types

    if "antenv.axon_hooks" not in sys.modules:
        mod = types.ModuleType("antenv.axon_hooks")
        _h = [None]
        mod.set_axon_ntff_profile_hook = lambda h: _h.__setitem__(0, h)
        mod.get_axon_ntff_profile_hook = lambda: _h[0]
        sys.modules["antenv.axon_hooks"] = mod
        import antenv

        antenv.axon_hooks = mod
    import antenv.axon_hooks as ah

    if ah.get_axon_ntff_profile_hook() is None:
        from trn_agent_boot.trn_boot import _ntff_profile_via_ctypes

        h = _ntff_profile_via_ctypes("/opt/axon/libaxon_pjrt.so")
        if h is not None:
            ah.set_axon_ntff_profile_hook(h)
    # avoid cloud artifact uploads in this container
    import concourse.bass_utils as bu

    bu.upload_artifacts = lambda tmpdir: tmpdir


def _run(x, trace=False, trace_kwargs=None):
    """x: (8, 2048, 4096) float32. Returns (out, exec_time_ns|None)."""
    x = np.ascontiguousarray(np.asarray(x, dtype=np.float32))
    assert x.shape == (N_CORES, 2048, 4096), x.shape
    nc = _get_nc()
    core_ids = list(range(N_CORES))
    in_maps = [{"x": x[i].reshape(NT, P, FD)} for i in range(N_CORES)]
    kwargs = {}
    if trace:
        _ensure_ntff_hook()
        kwargs["trace"] = True
        if trace_kwargs:
            kwargs.update(trace_kwargs)
    res = run_bass_kernel_spmd(nc, in_maps, core_ids, **kwargs)
    out = np.empty((N_CORES, 2048, 4096), dtype=np.float32)
    for i in range(N_CORES):
        out[i] = np.asarray(res.results[i]["out"], dtype=np.float32).reshape(
            2048, 4096
        )
    return out, res.exec_time_ns


def kernel(x, seg=None, silu_vals=None, **_unused):
    out, _ = _run(x, trace=False)
    return out
